# revision 14
# baseline (speedup 1.0000x reference)
"""BiLSTM (B=64, L=256, D=512, H=512) on 8 Trainium2 NeuronCores.

Sharding: 8 cores = 2 directions x 4 batch-slices of 16 (weights replicated
per direction, time loop local per core; backward cores get time-reversed x
so all cores run one SPMD program).

v5: two independent batch-8 chains per core, software-pipelined so each
chain's recurrence latency hides behind the other chain's work.

Per-core program:
  Phase 1 (interleaved): xpart[token, 4H] = x_t @ Wx.T + bias staged in SBUF.
  Per step t, per chain X in {A=batch 0:8, B=8:16}:
    - gates psum P_X (128,512): ident-inject xpart + 16 col-strip matmuls
      (hT_X stationary, Wh moving, 4 strips concurrent);
    - ONE sigmoid over strips f/i/o/g~ (g weights pre-scaled 2x so
      tanh(x) = 2*sig(2x)-1  =>  i*tanh = 2*i*(sig-0.5));
    - cell update as 4 DVE ops: r=(g~-0.5)*i [STT], q=f*c [TT],
      c=2r+q [STT], h=o*th [TT]; tanh on ScalarE;
    - hT via XBAR DMA-transpose (h [16,512] -> slot of a [128,4,8,16]
      ring) issued from SP, off the PE;
    - out DMA batched: one ring -> DRAM transfer per 8 steps (GpSimd),
      into a transposed DRAM layout [128,4,L,B] the host unpacks.
"""

import numpy as np
import ml_dtypes

from concourse import tile, mybir, bacc
from concourse.bass_utils import run_bass_kernel_spmd
from concourse.masks import make_identity

FP = mybir.dt.float32
BF = mybir.dt.bfloat16
AF = mybir.ActivationFunctionType
ALU = mybir.AluOpType

B = 16        # local batch per core
BC = 8        # batch per chain
L = 256       # timesteps
D = 512       # input dim
H = 512       # hidden
NG = 4 * H    # gate width
TOK = L * B   # tokens per core
NM = TOK // 128

_CACHED_NC = None


def _build():
    nc = bacc.Bacc("TRN2", target_bir_lowering=False, debug=False)

    xT = nc.dram_tensor("xT", [D, TOK], BF, kind="ExternalInput").ap()
    W = nc.dram_tensor("W", [D + H, NG], BF, kind="ExternalInput").ap()
    bias = nc.dram_tensor("bias", [1, NG], BF, kind="ExternalInput").ap()
    # transposed output layout: out_hT[p, k, t, b] = h_t[b, 128k+p]
    out_hT = nc.dram_tensor("out_hT", [128, 4, L, B], BF,
                            kind="ExternalOutput").ap()

    with tile.TileContext(nc, trace_sim=False) as tc:
        with tc.tile_pool(name="wpool", bufs=1) as wpool, \
             tc.tile_pool(name="cpool", bufs=1) as cpool:
            W_t = []
            for k in range(8):
                wt = wpool.tile([128, NG], BF, tag=f"w{k}", name=f"w{k}")
                nc.sync.dma_start(wt[:], W[128 * k:128 * (k + 1), :])
                W_t.append(wt)
            bias_t = wpool.tile([1, NG], BF)
            nc.sync.dma_start(bias_t[:], bias[:, :])
            ones_t = cpool.tile([1, 128], BF)
            nc.vector.memset(ones_t[:, :], 1.0)
            ident = cpool.tile([BC, BC], BF)
            make_identity(nc, ident[:, :])
            hT_zero = cpool.tile([128, 4, B], BF)
            nc.vector.memset(hT_zero[:, :, :], 0.0)

            with tc.tile_pool(name="p1x", bufs=3) as p1x, \
                 tc.tile_pool(name="xsp", bufs=5) as xsp, \
                 tc.tile_pool(name="p1ps", bufs=2, space="PSUM") as p1ps, \
                 tc.tile_pool(name="xpp", bufs=2) as xpp, \
                 tc.tile_pool(name="st", bufs=2) as st, \
                 tc.tile_pool(name="ch", bufs=2) as ch, \
                 tc.tile_pool(name="rng", bufs=2) as rng, \
                 tc.tile_pool(name="gpsA", bufs=2, space="PSUM") as gpsA, \
                 tc.tile_pool(name="gpsB", bufs=2, space="PSUM") as gpsB:
                gps = {0: gpsA, 1: gpsB}

                xps = {}
                p1n = [0]

                def emit_p1_part(m, n):
                    if n == 0:
                        xps[m] = xsp.tile([128, NG], BF, tag="xps",
                                          name=f"xps{m}")
                        xm = p1x.tile([128, 4, 128], BF, tag="xm", name="xm")
                        src = xT[:, 128 * m:128 * (m + 1)]
                        nc.gpsimd.dma_start(
                            xm[:, :, :],
                            src.rearrange("(k p) c -> p k c", k=4))
                        xps[m + 1000] = xm  # stash ref
                    xm = xps[m + 1000]
                    ps = p1ps.tile([128, 512], FP, tag="ps1", name="ps1")
                    for k in range(4):
                        nc.tensor.matmul(
                            ps[:, :], xm[:, k, :],
                            W_t[k][:, 512 * n:512 * (n + 1)],
                            start=(k == 0), stop=False)
                    nc.tensor.matmul(
                        ps[:, :], ones_t[:, :],
                        bias_t[:, 512 * n:512 * (n + 1)],
                        start=False, stop=True)
                    # psum->sbuf stage; alternate ScalarE/DVE (GpSimd
                    # cannot access PSUM)
                    if p1n[0] % 2 == 0:
                        nc.scalar.copy(
                            xps[m][:, 512 * n:512 * (n + 1)], ps[:, :])
                    else:
                        nc.vector.tensor_copy(
                            xps[m][:, 512 * n:512 * (n + 1)], ps[:, :])
                    p1n[0] += 1

                def emit_p1(m):
                    for n in range(4):
                        emit_p1_part(m, n)

                # per-chain state
                c_prev = []
                for X in range(2):
                    c0 = st.tile([BC, H], BF, tag=f"c{X}", name=f"c{X}_0")
                    nc.vector.memset(c0[:, :], 0.0)
                    c_prev.append(c0)
                hT_prev = [(hT_zero, None), (hT_zero, None)]
                ring = [None, None]
                xp_t = {}

                def emit_xp(t):
                    # per-chain xpart slice staged at partition base 0
                    # (matmul moving operands must start at partition 0)
                    for X in range(2):
                        xp = xpp.tile([BC, NG], BF, tag=f"xp{X}",
                                      name=f"xp{X}")
                        r0 = B * (t % 8) + BC * X
                        nc.gpsimd.dma_start(
                            xp[:], xps[t // 8][r0:r0 + BC, :])
                        xp_t[(t, X)] = xp

                for m in range(2):
                    emit_p1(m)
                emit_xp(0)
                emit_xp(1)

                for t in range(L):
                    if t % 2 == 0 and t // 8 + 2 < NM:
                        emit_p1_part(t // 8 + 2, (t % 8) // 2)
                    if t % 8 == 0:
                        for X in range(2):
                            ring[X] = rng.tile([128, 4, 8, B], BF,
                                               tag=f"ring{X}",
                                               name=f"ring{X}_{t}")

                    # --- PE: gates for both chains ---
                    P = []
                    for X in range(2):
                        PX = gps[X].tile([128, 512], FP, tag="P",
                                         name=f"P{X}")
                        P.append(PX)
                        xp = xp_t.pop((t, X))
                        for j in range(4):
                            nc.tensor.matmul(
                                PX[32 * j:32 * j + BC, :], ident[:, :],
                                xp[:, 512 * j:512 * (j + 1)],
                                start=True, stop=False,
                                tile_position=(0, 32 * j))
                        hTp, slot = hT_prev[X]
                        for k in range(4):
                            if slot is None:
                                lhs = hTp[:, k, 0:BC]
                            else:
                                lhs = hTp[:, k, slot, 0:BC]
                            for j in range(4):
                                nc.tensor.matmul(
                                    PX[32 * j:32 * j + BC, :], lhs,
                                    W_t[4 + k][:, 512 * j:512 * (j + 1)],
                                    start=False, stop=(k == 3),
                                    tile_position=(0, 32 * j))

                    # --- ScalarE: one sigmoid per chain over f/i/o/g~ ---
                    s = []
                    for X in range(2):
                        sX = ch.tile([112, H], BF, tag=f"s{X}", name=f"s{X}")
                        nc.scalar.activation(sX[:, :], P[X][0:112, :],
                                             AF.Sigmoid)
                        s.append(sX)

                    # --- cell update on DVE ---
                    # u' = g~-0.5 (staged at rows 32 to pair with i strip);
                    # r = i*u' ; q = f*c ; c' = 2r + q
                    u_, r_, q_ = [], [], []
                    for X in range(2):
                        uX = ch.tile([32 + BC, H], BF, tag=f"u{X}",
                                     name=f"u{X}")
                        nc.vector.tensor_scalar_add(
                            uX[32:32 + BC, :], s[X][96:96 + BC, :], -0.5)
                        u_.append(uX)
                    for X in range(2):
                        rX = ch.tile([BC, H], BF, tag=f"r{X}", name=f"r{X}")
                        nc.vector.tensor_mul(
                            rX[:, :], s[X][32:32 + BC, :],
                            u_[X][32:32 + BC, :])
                        r_.append(rX)
                    for X in range(2):
                        qX = ch.tile([BC, H], BF, tag=f"q{X}", name=f"q{X}")
                        nc.vector.tensor_mul(qX[:, :], s[X][0:BC, :],
                                             c_prev[X][:, :])
                        q_.append(qX)
                    c_new = []
                    for X in range(2):
                        cX = st.tile([BC, H], BF, tag=f"c{X}", name=f"c{X}")
                        nc.vector.scalar_tensor_tensor(
                            cX[:, :], r_[X][:, :], 2.0, q_[X][:, :],
                            op0=ALU.mult, op1=ALU.add)
                        c_new.append(cX)
                    # th at rows 64:72 to pair with the o strip for hmul
                    th = []
                    for X in range(2):
                        thX = ch.tile([64 + BC, H], BF, tag=f"th{X}",
                                      name=f"th{X}")
                        nc.scalar.activation(thX[64:64 + BC, :],
                                             c_new[X][:, :], AF.Tanh)
                        th.append(thX)
                    h = []
                    for X in range(2):
                        hX = st.tile([B, H], BF, tag=f"h{X}", name=f"h{X}")
                        nc.vector.tensor_mul(hX[0:BC, :], s[X][64:64 + BC, :],
                                             th[X][64:64 + BC, :])
                        h.append(hX)

                    # --- hT via XBAR dma transpose into ring slot (SP) ---
                    for X in range(2):
                        nc.sync.dma_start(ring[X][:, :, t % 8, :],
                                          h[X][:, :], transpose=True)
                        hT_prev[X] = (ring[X], t % 8)

                    # --- batched out DMA once per 8 steps (GpSimd) ---
                    if t % 8 == 7:
                        t0 = t - 7
                        for X in range(2):
                            nc.gpsimd.dma_start(
                                out_hT[:, :, t0:t0 + 8, BC * X:BC * X + BC],
                                ring[X][:, :, :, 0:BC])
                    if t + 2 < L:
                        emit_xp(t + 2)

                    c_prev = c_new
    nc.compile()
    return nc


def _host_prepare(x_full, weights, direction, bslice):
    xs = x_full[bslice]
    if direction == "bw":
        xs = xs[:, ::-1, :]
    xT = np.ascontiguousarray(xs.transpose(2, 1, 0).reshape(D, TOK))
    Wc = np.concatenate(
        [weights[f"W_{direction}_{n}"].T for n in "fiog"], axis=1).copy()
    bc = np.concatenate(
        [weights[f"b_{direction}_{n}"] for n in "fiog"])[None, :].copy()
    # tanh fold: g strip pre-activations scaled by 2 (tanh(x) = 2*sig(2x)-1)
    Wc[:, 3 * H:] *= 2.0
    bc[:, 3 * H:] *= 2.0
    return {"xT": np.ascontiguousarray(xT).astype(ml_dtypes.bfloat16),
            "W": np.ascontiguousarray(Wc).astype(ml_dtypes.bfloat16),
            "bias": np.ascontiguousarray(bc).astype(ml_dtypes.bfloat16)}


def kernel(**inputs):
    global _CACHED_NC
    inputs = {k: np.asarray(v) for k, v in inputs.items()}
    x = inputs["x"]
    Bx, Lx, _ = x.shape
    assert (Bx, Lx) == (64, L)

    if _CACHED_NC is None:
        _CACHED_NC = _build()
    nc = _CACHED_NC

    in_maps = []
    meta = []
    for ci in range(8):
        d = "fw" if ci < 4 else "bw"
        bs = (ci % 4) * B
        in_maps.append(_host_prepare(x, inputs, d, slice(bs, bs + B)))
        meta.append((d, bs))

    res = run_bass_kernel_spmd(nc, in_maps, core_ids=list(range(8)))

    hf = np.zeros((L, Bx, H), np.float32)
    hb = np.zeros((L, Bx, H), np.float32)
    for ci in range(8):
        d, bs = meta[ci]
        # out_hT[p, k, t, b] = h_t[b, 128k+p] -> oh[t, b, j]
        ohT = np.asarray(res.results[ci]["out_hT"]).astype(np.float32)
        oh = ohT.transpose(2, 3, 1, 0).reshape(L, B, H)
        if d == "fw":
            hf[:, bs:bs + B, :] = oh
        else:
            hb[:, bs:bs + B, :] = oh[::-1]

    # faithful to the reference: stack time-major, flatten, hstack, reshape
    flat = np.concatenate([hf.reshape(-1, H), hb.reshape(-1, H)], axis=1)
    return flat.reshape(Bx, Lx, 2 * H).astype(np.float32)


# revision 23
# speedup vs baseline: 1.5809x; 1.5809x over previous
"""BiLSTM (B=64, L=256, D=512, H=512) on 8 Trainium2 NeuronCores.

Sharding: 8 cores = 2 directions x 4 batch-slices of 16 (weights replicated
per direction, time loop local per core; backward cores get time-reversed x
so all cores run one SPMD program).

v5: two independent batch-8 chains per core, software-pipelined so each
chain's recurrence latency hides behind the other chain's work.

Per-core program:
  Phase 1 (interleaved): xpart[token, 4H] = x_t @ Wx.T + bias staged in SBUF.
  Per step t, per chain X in {A=batch 0:8, B=8:16}:
    - gates psum P_X (128,512): ident-inject xpart + 16 col-strip matmuls
      (hT_X stationary, Wh moving, 4 strips concurrent);
    - ONE sigmoid over strips f/i/o/g~ (g weights pre-scaled 2x so
      tanh(x) = 2*sig(2x)-1  =>  i*tanh = 2*i*(sig-0.5));
    - cell update: u=2g~-1, t2=i*u, q=f*c, c=q+t2 on DVE; tanh on
      ScalarE; h=o*th split halves across DVE and GpSimd;
    - hT via 4 PE transposes into a psum strip, one copy into a
      [128,8,4,8] SBUF ring slot (copies alternate ScalarE/DVE);
    - out DMA batched: one ring -> DRAM transfer per 8 steps (GpSimd),
      into a transposed DRAM layout [128,4,L,B] the host unpacks.
"""

import numpy as np
import ml_dtypes

from concourse import tile, mybir, bacc
from concourse.bass_utils import run_bass_kernel_spmd
from concourse.masks import make_identity

FP = mybir.dt.float32
BF = mybir.dt.bfloat16
AF = mybir.ActivationFunctionType
ALU = mybir.AluOpType

B = 16        # local batch per core
BC = 8        # batch per chain
L = 256       # timesteps
D = 512       # input dim
H = 512       # hidden
NG = 4 * H    # gate width
TOK = L * B   # tokens per core
NM = TOK // 128

_CACHED_NC = None


def _build():
    nc = bacc.Bacc("TRN2", target_bir_lowering=False, debug=False)

    xT = nc.dram_tensor("xT", [D, TOK], BF, kind="ExternalInput").ap()
    W = nc.dram_tensor("W", [D + H, NG], BF, kind="ExternalInput").ap()
    bias = nc.dram_tensor("bias", [1, NG], BF, kind="ExternalInput").ap()
    # transposed output layout: out_hT[p, t, k, b] = h_t[b, 128k+p]
    out_hT = nc.dram_tensor("out_hT", [128, L, 4, B], BF,
                            kind="ExternalOutput").ap()

    with tile.TileContext(nc, trace_sim=False) as tc:
        with tc.tile_pool(name="wpool", bufs=1) as wpool, \
             tc.tile_pool(name="cpool", bufs=1) as cpool:
            W_t = []
            for k in range(8):
                wt = wpool.tile([128, NG], BF, tag=f"w{k}", name=f"w{k}")
                nc.sync.dma_start(wt[:], W[128 * k:128 * (k + 1), :])
                W_t.append(wt)
            bias_t = wpool.tile([1, NG], BF)
            nc.sync.dma_start(bias_t[:], bias[:, :])
            ones_t = cpool.tile([1, 128], BF)
            nc.vector.memset(ones_t[:, :], 1.0)
            ident = cpool.tile([BC, BC], BF)
            make_identity(nc, ident[:, :])
            hT_zero = cpool.tile([128, 4, B], BF)
            nc.vector.memset(hT_zero[:, :, :], 0.0)

            with tc.tile_pool(name="p1x", bufs=3) as p1x, \
                 tc.tile_pool(name="xsp", bufs=5) as xsp, \
                 tc.tile_pool(name="p1ps", bufs=2, space="PSUM") as p1ps, \
                 tc.tile_pool(name="xpp", bufs=2) as xpp, \
                 tc.tile_pool(name="st", bufs=2) as st, \
                 tc.tile_pool(name="ch", bufs=2) as ch, \
                 tc.tile_pool(name="rng", bufs=2) as rng, \
                 tc.tile_pool(name="tps", bufs=1, space="PSUM") as tps, \
                 tc.tile_pool(name="gpsA", bufs=2, space="PSUM") as gpsA, \
                 tc.tile_pool(name="gpsB", bufs=2, space="PSUM") as gpsB:
                gps = {0: gpsA, 1: gpsB}

                xps = {}
                p1n = [0]

                def emit_p1_part(m, n):
                    if n == 0:
                        xps[m] = xsp.tile([128, NG], BF, tag="xps",
                                          name=f"xps{m}")
                        xm = p1x.tile([128, 4, 128], BF, tag="xm", name="xm")
                        src = xT[:, 128 * m:128 * (m + 1)]
                        nc.gpsimd.dma_start(
                            xm[:, :, :],
                            src.rearrange("(k p) c -> p k c", k=4))
                        xps[m + 1000] = xm  # stash ref
                    xm = xps[m + 1000]
                    ps = p1ps.tile([128, 512], FP, tag="ps1", name="ps1")
                    for k in range(4):
                        nc.tensor.matmul(
                            ps[:, :], xm[:, k, :],
                            W_t[k][:, 512 * n:512 * (n + 1)],
                            start=(k == 0), stop=False)
                    nc.tensor.matmul(
                        ps[:, :], ones_t[:, :],
                        bias_t[:, 512 * n:512 * (n + 1)],
                        start=False, stop=True)
                    # psum->sbuf stage; alternate ScalarE/DVE (GpSimd
                    # cannot access PSUM)
                    if p1n[0] % 2 == 0:
                        nc.scalar.copy(
                            xps[m][:, 512 * n:512 * (n + 1)], ps[:, :])
                    else:
                        nc.vector.tensor_copy(
                            xps[m][:, 512 * n:512 * (n + 1)], ps[:, :])
                    p1n[0] += 1

                def emit_p1(m):
                    for n in range(4):
                        emit_p1_part(m, n)

                # per-chain state
                c_prev = []
                for X in range(2):
                    c0 = st.tile([BC, H], BF, tag=f"c{X}", name=f"c{X}_0")
                    nc.vector.memset(c0[:, :], 0.0)
                    c_prev.append(c0)
                hT_prev = [(hT_zero, None), (hT_zero, None)]
                ring = [None, None]
                xp_t = {}

                def emit_xp(t):
                    # per-chain xpart slice staged at partition base 0
                    # (matmul moving operands must start at partition 0)
                    for X in range(2):
                        xp = xpp.tile([BC, NG], BF, tag=f"xp{X}",
                                      name=f"xp{X}")
                        r0 = B * (t % 8) + BC * X
                        nc.sync.dma_start(
                            xp[:], xps[t // 8][r0:r0 + BC, :])
                        xp_t[(t, X)] = xp

                for m in range(2):
                    emit_p1(m)
                emit_xp(0)
                emit_xp(1)

                for t in range(L):
                    if t % 2 == 0 and t // 8 + 2 < NM:
                        emit_p1_part(t // 8 + 2, (t % 8) // 2)
                    if t % 8 == 0:
                        for X in range(2):
                            ring[X] = rng.tile([128, 8, 4, BC], BF,
                                               tag=f"ring{X}",
                                               name=f"ring{X}_{t}")

                    # --- PE: gates for both chains ---
                    P = []
                    for X in range(2):
                        PX = gps[X].tile([128, 512], FP, tag="P",
                                         name=f"P{X}")
                        P.append(PX)
                        xp = xp_t.pop((t, X))
                        for j in range(4):
                            nc.tensor.matmul(
                                PX[32 * j:32 * j + BC, :], ident[:, :],
                                xp[:, 512 * j:512 * (j + 1)],
                                start=True, stop=False,
                                tile_position=(0, 32 * j))
                        hTp, slot = hT_prev[X]
                        for k in range(4):
                            if slot is None:
                                lhs = hTp[:, k, 0:BC]
                            else:
                                lhs = hTp[:, slot, k, :]
                            for j in range(4):
                                nc.tensor.matmul(
                                    PX[32 * j:32 * j + BC, :], lhs,
                                    W_t[4 + k][:, 512 * j:512 * (j + 1)],
                                    start=False, stop=(k == 3),
                                    tile_position=(0, 32 * j))

                    # --- ScalarE: one sigmoid per chain over f/i/o/g~ ---
                    s = []
                    for X in range(2):
                        sX = ch.tile([112, H], BF, tag=f"s{X}", name=f"s{X}")
                        nc.scalar.activation(sX[:, :], P[X][0:112, :],
                                             AF.Sigmoid)
                        s.append(sX)

                    # --- cell update on DVE ---
                    # u = 2*g~-1 = tanh (staged at rows 32 to pair with i);
                    # t2 = i*u ; q = f*c ; c' = q + t2
                    u_, r_, q_ = [], [], []
                    for X in range(2):
                        uX = ch.tile([32 + BC, H], BF, tag=f"u{X}",
                                     name=f"u{X}")
                        nc.vector.tensor_scalar(
                            uX[32:32 + BC, :], s[X][96:96 + BC, :], 2.0, -1.0,
                            op0=ALU.mult, op1=ALU.add)
                        u_.append(uX)
                    for X in range(2):
                        rX = ch.tile([BC, H], BF, tag=f"t2{X}",
                                     name=f"t2{X}")
                        nc.vector.tensor_mul(
                            rX[:, :], s[X][32:32 + BC, :],
                            u_[X][32:32 + BC, :])
                        r_.append(rX)
                    for X in range(2):
                        qX = ch.tile([BC, H], BF, tag=f"q{X}", name=f"q{X}")
                        nc.vector.tensor_mul(qX[:, :], s[X][0:BC, :],
                                             c_prev[X][:, :])
                        q_.append(qX)
                    c_new = []
                    for X in range(2):
                        cX = st.tile([BC, H], BF, tag=f"c{X}", name=f"c{X}")
                        nc.vector.tensor_add(cX[:, :], q_[X][:, :],
                                             r_[X][:, :])
                        c_new.append(cX)
                    # th at rows 64:72 to pair with the o strip for hmul
                    th = []
                    for X in range(2):
                        thX = ch.tile([64 + BC, H], BF, tag=f"th{X}",
                                      name=f"th{X}")
                        nc.scalar.activation(thX[64:64 + BC, :],
                                             c_new[X][:, :], AF.Tanh)
                        th.append(thX)
                    # h = o*th, halves split DVE / GpSimd
                    h = []
                    for X in range(2):
                        hX = st.tile([B, H], BF, tag=f"h{X}", name=f"h{X}")
                        nc.vector.tensor_mul(
                            hX[0:BC, 0:256], s[X][64:64 + BC, 0:256],
                            th[X][64:64 + BC, 0:256])
                        nc.gpsimd.tensor_mul(
                            hX[0:BC, 256:512], s[X][64:64 + BC, 256:512],
                            th[X][64:64 + BC, 256:512])
                        h.append(hX)

                    # --- hT: 4 PE transposes -> psum strip -> ring slot ---
                    for X in range(2):
                        tp = tps.tile([128, 4 * BC], BF, tag=f"tp{X}",
                                      name=f"tp{X}")
                        for k in range(4):
                            nc.tensor.transpose(
                                tp[:, BC * k:BC * (k + 1)],
                                h[X][0:BC, 128 * k:128 * (k + 1)],
                                ident[:, :])
                        if X == 0:
                            nc.scalar.copy(ring[X][:, t % 8, :, :], tp[:, :])
                        else:
                            nc.vector.tensor_copy(ring[X][:, t % 8, :, :],
                                                  tp[:, :])
                        hT_prev[X] = (ring[X], t % 8)

                    # --- batched out DMA once per 8 steps (GpSimd) ---
                    if t % 8 == 7:
                        t0 = t - 7
                        for X in range(2):
                            nc.gpsimd.dma_start(
                                out_hT[:, t0:t0 + 8, :, BC * X:BC * X + BC],
                                ring[X][:, :, :, :])
                    if t + 2 < L:
                        emit_xp(t + 2)

                    c_prev = c_new
    nc.compile()
    return nc


def _host_prepare(x_full, weights, direction, bslice):
    xs = x_full[bslice]
    if direction == "bw":
        xs = xs[:, ::-1, :]
    xT = np.ascontiguousarray(xs.transpose(2, 1, 0).reshape(D, TOK))
    Wc = np.concatenate(
        [weights[f"W_{direction}_{n}"].T for n in "fiog"], axis=1).copy()
    bc = np.concatenate(
        [weights[f"b_{direction}_{n}"] for n in "fiog"])[None, :].copy()
    # tanh fold: g strip pre-activations scaled by 2 (tanh(x) = 2*sig(2x)-1)
    Wc[:, 3 * H:] *= 2.0
    bc[:, 3 * H:] *= 2.0
    return {"xT": np.ascontiguousarray(xT).astype(ml_dtypes.bfloat16),
            "W": np.ascontiguousarray(Wc).astype(ml_dtypes.bfloat16),
            "bias": np.ascontiguousarray(bc).astype(ml_dtypes.bfloat16)}


def kernel(**inputs):
    global _CACHED_NC
    inputs = {k: np.asarray(v) for k, v in inputs.items()}
    x = inputs["x"]
    Bx, Lx, _ = x.shape
    assert (Bx, Lx) == (64, L)

    if _CACHED_NC is None:
        _CACHED_NC = _build()
    nc = _CACHED_NC

    in_maps = []
    meta = []
    for ci in range(8):
        d = "fw" if ci < 4 else "bw"
        bs = (ci % 4) * B
        in_maps.append(_host_prepare(x, inputs, d, slice(bs, bs + B)))
        meta.append((d, bs))

    res = run_bass_kernel_spmd(nc, in_maps, core_ids=list(range(8)))

    hf = np.zeros((L, Bx, H), np.float32)
    hb = np.zeros((L, Bx, H), np.float32)
    for ci in range(8):
        d, bs = meta[ci]
        # out_hT[p, t, k, b] = h_t[b, 128k+p] -> oh[t, b, j]
        ohT = np.asarray(res.results[ci]["out_hT"]).astype(np.float32)
        oh = ohT.transpose(1, 3, 2, 0).reshape(L, B, H)
        if d == "fw":
            hf[:, bs:bs + B, :] = oh
        else:
            hb[:, bs:bs + B, :] = oh[::-1]

    # faithful to the reference: stack time-major, flatten, hstack, reshape
    flat = np.concatenate([hf.reshape(-1, H), hb.reshape(-1, H)], axis=1)
    return flat.reshape(Bx, Lx, 2 * H).astype(np.float32)


# revision 24
# speedup vs baseline: 2.8713x; 1.8163x over previous
"""BiLSTM (B=64, L=256, D=512, H=512) on 8 Trainium2 NeuronCores.

Strategy: 8 cores = 2 directions x 4 batch-slices of 16 (weights replicated
per direction, sequential time loop local to each core — no cross-core
communication).  Backward-direction cores receive time-reversed x, so every
core runs the identical SPMD program.

v7 (surgical over the v2 baseline):
  - bf16 matmuls, fp32 PSUM; g-gate weights pre-scaled 2x so ONE sigmoid
    covers all four gate strips (tanh(x) = 2*sig(2x)-1).
  - single full-width tanh / h-mul / out-DMA per step (baseline split them
    into halves);
  - all 4 h-transposes land in ONE psum tile, moved by ONE DVE copy;
  - xpart-slice staging and out_h DMAs issued from GpSimd, phase-1 x
    tile loads as one rearranged DMA from GpSimd (keeps SP/ACT free);
  - phase-1 GEMM interleaved with the recurrence for PE warmth.
"""

import numpy as np
import ml_dtypes

from concourse import tile, mybir, bacc
from concourse.bass_utils import run_bass_kernel_spmd
from concourse.masks import make_identity

FP = mybir.dt.float32
BF = mybir.dt.bfloat16
AF = mybir.ActivationFunctionType
ALU = mybir.AluOpType

B = 16        # local batch per core
L = 256       # timesteps
D = 512       # input dim
H = 512       # hidden
NG = 4 * H    # gate width
TOK = L * B   # tokens per core
NM = TOK // 128

_CACHED_NC = None


def _build():
    nc = bacc.Bacc("TRN2", target_bir_lowering=False, debug=False)

    xT = nc.dram_tensor("xT", [D, TOK], BF, kind="ExternalInput").ap()
    W = nc.dram_tensor("W", [D + H, NG], BF, kind="ExternalInput").ap()
    bias = nc.dram_tensor("bias", [1, NG], BF, kind="ExternalInput").ap()
    out_h = nc.dram_tensor("out_h", [L, B, H], BF, kind="ExternalOutput").ap()

    with tile.TileContext(nc, trace_sim=False) as tc:
        with tc.tile_pool(name="wpool", bufs=1) as wpool, \
             tc.tile_pool(name="cpool", bufs=1) as cpool:
            W_t = []
            for k in range(8):
                wt = wpool.tile([128, NG], BF, tag=f"w{k}", name=f"w{k}")
                nc.sync.dma_start(wt[:], W[128 * k:128 * (k + 1), :])
                W_t.append(wt)
            bias_t = wpool.tile([1, NG], BF)
            nc.sync.dma_start(bias_t[:], bias[:, :])
            ones_t = cpool.tile([1, 128], BF)
            nc.vector.memset(ones_t[:, :], 1.0)
            ident = cpool.tile([B, B], BF)
            make_identity(nc, ident[:, :])

            with tc.tile_pool(name="p1x", bufs=3) as p1x, \
                 tc.tile_pool(name="xsp", bufs=5) as xsp, \
                 tc.tile_pool(name="p1ps", bufs=2, space="PSUM") as p1ps, \
                 tc.tile_pool(name="xpp", bufs=3) as xpp, \
                 tc.tile_pool(name="st", bufs=2) as st, \
                 tc.tile_pool(name="ch", bufs=2) as ch, \
                 tc.tile_pool(name="gps", bufs=2, space="PSUM") as gps, \
                 tc.tile_pool(name="tps", bufs=2, space="PSUM") as tps:

                xps = {}
                xm_map = {}
                p1n = [0]

                def emit_p1_part(m, n):
                    if n == 0:
                        xps[m] = xsp.tile([128, NG], BF, tag="xps",
                                          name=f"xps{m}")
                        xm = p1x.tile([128, 4, 128], BF, tag="xm", name="xm")
                        src = xT[:, 128 * m:128 * (m + 1)]
                        nc.gpsimd.dma_start(
                            xm[:, :, :],
                            src.rearrange("(k p) c -> p k c", k=4))
                        xm_map[m] = xm
                    xm = xm_map[m]
                    ps = p1ps.tile([128, 512], FP, tag="ps1", name="ps1")
                    for k in range(4):
                        nc.tensor.matmul(
                            ps[:, :], xm[:, k, :],
                            W_t[k][:, 512 * n:512 * (n + 1)],
                            start=(k == 0), stop=False)
                    nc.tensor.matmul(
                        ps[:, :], ones_t[:, :],
                        bias_t[:, 512 * n:512 * (n + 1)],
                        start=False, stop=True)
                    # psum->sbuf stage; alternate ScalarE/DVE
                    if p1n[0] % 2 == 0:
                        nc.scalar.copy(
                            xps[m][:, 512 * n:512 * (n + 1)], ps[:, :])
                    else:
                        nc.vector.tensor_copy(
                            xps[m][:, 512 * n:512 * (n + 1)], ps[:, :])
                    p1n[0] += 1

                def emit_p1(m):
                    for n in range(4):
                        emit_p1_part(m, n)

                xp_t = {}

                def emit_xp(t):
                    xp = xpp.tile([B, NG], BF, tag="xp", name="xp")
                    nc.gpsimd.dma_start(
                        xp[:], xps[t // 8][B * (t % 8):B * (t % 8) + B, :])
                    xp_t[t] = xp

                c_prev = st.tile([B, H], BF, tag="c", name="c0")
                nc.vector.memset(c_prev[:, :], 0.0)
                hT_prev = st.tile([128, 4, B], BF, tag="hT", name="hT0")
                nc.vector.memset(hT_prev[:, :, :], 0.0)

                for m in range(2):
                    emit_p1(m)
                emit_xp(0)
                emit_xp(1)

                for t in range(L):
                    if t % 2 == 0 and t // 8 + 2 < NM:
                        emit_p1_part(t // 8 + 2, (t % 8) // 2)

                    xp = xp_t.pop(t)
                    P = gps.tile([128, 512], FP, tag="P", name="P")
                    for j in range(4):
                        nc.tensor.matmul(
                            P[32 * j:32 * j + B, :], ident[:, :],
                            xp[:, 512 * j:512 * (j + 1)],
                            start=True, stop=False, tile_position=(0, 32 * j))
                    for k in range(4):
                        for j in range(4):
                            nc.tensor.matmul(
                                P[32 * j:32 * j + B, :],
                                hT_prev[:, k, :],
                                W_t[4 + k][:, 512 * j:512 * (j + 1)],
                                start=False, stop=(k == 3),
                                tile_position=(0, 32 * j))

                    # strips: f@0:16, i@32:48, o@64:80, g~@96:112
                    s = ch.tile([112, H], BF, tag="s", name="s")
                    nc.scalar.activation(s[:, :], P[0:112, :], AF.Sigmoid)
                    # u = 2*g~ - 1 = tanh(x_g), at rows 32:48 to align with i
                    u = ch.tile([48, H], BF, tag="u", name="u")
                    nc.vector.tensor_scalar(
                        u[32:48, :], s[96:112, :], 2.0, -1.0,
                        op0=ALU.mult, op1=ALU.add)
                    t1 = ch.tile([B, H], BF, tag="t1", name="t1")
                    nc.vector.tensor_mul(t1[:, :], s[0:B, :], c_prev[:, :])
                    t2 = ch.tile([B, H], BF, tag="t2", name="t2")
                    nc.vector.tensor_mul(t2[:, :], s[32:48, :], u[32:48, :])
                    c_new = st.tile([B, H], BF, tag="c", name="c")
                    nc.vector.tensor_add(c_new[:, :], t1[:, :], t2[:, :])
                    # th at rows 64:80 to align with the o strip
                    th = ch.tile([80, H], BF, tag="th", name="th")
                    nc.scalar.activation(th[64:80, :], c_new[:, :], AF.Tanh)
                    h = st.tile([B, H], BF, tag="h", name="h")
                    nc.vector.tensor_mul(h[:, :], s[64:80, :], th[64:80, :])

                    nc.gpsimd.dma_start(out_h[t, :, :], h[:, :])

                    # 4 PE transposes into one psum strip, one DVE copy out
                    tp = tps.tile([128, 4, B], BF, tag="tp", name="tp")
                    for k in range(4):
                        nc.tensor.transpose(
                            tp[:, k, :], h[:, 128 * k:128 * (k + 1)],
                            ident[:, :])
                    hT_new = st.tile([128, 4, B], BF, tag="hT", name="hT")
                    nc.vector.tensor_copy(hT_new[:, :, :], tp[:, :, :])

                    if t + 2 < L:
                        emit_xp(t + 2)

                    c_prev = c_new
                    hT_prev = hT_new
    nc.compile()
    return nc


def _host_prepare(x_full, weights, direction, bslice):
    xs = x_full[bslice]
    if direction == "bw":
        xs = xs[:, ::-1, :]
    xT = np.ascontiguousarray(xs.transpose(2, 1, 0).reshape(D, TOK))
    Wc = np.concatenate(
        [weights[f"W_{direction}_{n}"].T for n in "fiog"], axis=1).copy()
    bc = np.concatenate(
        [weights[f"b_{direction}_{n}"] for n in "fiog"])[None, :].copy()
    # tanh fold: g strip pre-activations scaled by 2 (tanh(x) = 2*sig(2x)-1)
    Wc[:, 3 * H:] *= 2.0
    bc[:, 3 * H:] *= 2.0
    return {"xT": np.ascontiguousarray(xT).astype(ml_dtypes.bfloat16),
            "W": np.ascontiguousarray(Wc).astype(ml_dtypes.bfloat16),
            "bias": np.ascontiguousarray(bc).astype(ml_dtypes.bfloat16)}


def kernel(**inputs):
    global _CACHED_NC
    inputs = {k: np.asarray(v) for k, v in inputs.items()}
    x = inputs["x"]
    Bx, Lx, _ = x.shape
    assert (Bx, Lx) == (64, L)

    if _CACHED_NC is None:
        _CACHED_NC = _build()
    nc = _CACHED_NC

    in_maps = []
    meta = []
    for ci in range(8):
        d = "fw" if ci < 4 else "bw"
        bs = (ci % 4) * B
        in_maps.append(_host_prepare(x, inputs, d, slice(bs, bs + B)))
        meta.append((d, bs))

    res = run_bass_kernel_spmd(nc, in_maps, core_ids=list(range(8)))

    hf = np.zeros((L, Bx, H), np.float32)
    hb = np.zeros((L, Bx, H), np.float32)
    for ci in range(8):
        d, bs = meta[ci]
        oh = np.asarray(res.results[ci]["out_h"]).astype(np.float32)
        if d == "fw":
            hf[:, bs:bs + B, :] = oh
        else:
            hb[:, bs:bs + B, :] = oh[::-1]

    # faithful to the reference: stack time-major, flatten, hstack, reshape
    flat = np.concatenate([hf.reshape(-1, H), hb.reshape(-1, H)], axis=1)
    return flat.reshape(Bx, Lx, 2 * H).astype(np.float32)
